# revision 10
# baseline (speedup 1.0000x reference)
"""Trainium2 Bass kernel for nn_Cross_attention_2 (sparse_attention).

Math (B=1, C=32, D=36, H=W=48, P=9):
  xc = conv1x1(x, W_img, b_img)            # per-voxel channel mix (bias deferred)
  v  = unfold(xc)                          # (C, L=1024, 81) non-overlapping 9x9 patches
  px = LeakyReLU(v @ (W2@W1)^T + bias)     # the two Linears collapse to A = W2@W1
  att[c] = px[c] @ py[c]^T / 81            # (C, 1024, 1024)

Sharding: channels C=32 split across 8 cores (4 each). Params replicated
(per-core slices precomputed on host). Each core reads full x, y.

Per-core device pipeline, all data fp16 (fp32 PSUM accumulation):
  load:      x, y cast to fp16 on host; per-(t,pd) loads with c-major outer AP
             so descriptors spread across the 16 SDMA engines
  conv:      3 accumulating matmuls (K=128/128/32) -> psum (36,512) chunks
             -> xc3 rows 0-35 (rows kd*4+o); conv bias NOT added here
  shift:     SBUF->SBUF DMAs replicate xc3 into rows 36-71 (cols shifted -3)
             and 72-107 (shifted -6) so one transform pass covers 3 kw values
  transform: 3 matmul passes (K=108) per output tile; epilogue is one scalar
             Lrelu activation: out = LeakyReLU(psum/9 + b_c*rowsum(A)/9)
             (1/9 per side => att carries the full 1/81)
  att:       pxT (81,1024) per (t,c); out tiles (128,512); fp16 store,
             host casts back to fp32
"""

import sys

sys.path.insert(0, "/opt/trn_rl_repo")

import contextlib
import os

import numpy as np

import concourse.bass as bass  # noqa: F401
import concourse.tile as tile
from concourse import bacc, mybir
from concourse.bass_utils import run_bass_kernel_spmd

P = 9
P2 = 81
C = 32
D = 36
HWF = 2304
ND = 4  # pd blocks (D/9)
L = 1024
N_CORES = 8
CPC = 4  # channels per core

F32 = mybir.dt.float32
F32R = mybir.dt.float32r
F16 = mybir.dt.float16

_CACHE = {}
last_results = None  # BassKernelResults of the most recent run (for test.py)

_HW_CHUNKS = [(0, 512), (512, 512), (1024, 512), (1536, 512), (2048, 256)]
IDENT = mybir.ActivationFunctionType.Identity


def _build():
    if "nc" in _CACHE:
        return _CACHE["nc"]

    nc = bacc.Bacc("TRN2", target_bir_lowering=False, debug=False,
                   num_devices=N_CORES)
    x_d = nc.dram_tensor("x", (C, D, HWF), F16, kind="ExternalInput").ap()
    y_d = nc.dram_tensor("y", (C, D, HWF), F16, kind="ExternalInput").ap()
    # wblk: conv lhsT, rows p = c*4+kd_l; col blocks (t*2+i)*32 + kd*4+o
    # for the kd 0-7 passes, then [128+t*16 +: 16] = kd8 block rows p=c*4+pd
    wblk_d = nc.dram_tensor("wblk", (128, 160), F16, kind="ExternalInput").ap()
    # tm3: (108, t*4c*3p*81j) transform weights, rows g*36 + kd*4 + o
    tm_d = nc.dram_tensor("tm", (108, 2 * CPC * 3 * P2), F32R,
                          kind="ExternalInput").ap()
    # biasc[r, t]: rows 0-31 = b_t[r%4] (kd*4+o), rows 32-47 = b_t[r%4] (pd*4+o)
    biasc_d = nc.dram_tensor("biasc", (48, 2), F32, kind="ExternalInput").ap()
    att_d = nc.dram_tensor("att", (CPC, L, L), F16, kind="ExternalOutput").ap()

    with tile.TileContext(nc) as tc:
        with contextlib.ExitStack() as ctx:
            consts = ctx.enter_context(tc.tile_pool(name="consts", bufs=1))
            xbp = ctx.enter_context(tc.tile_pool(name="xb", bufs=6))
            xb8p = ctx.enter_context(tc.tile_pool(name="xb8", bufs=2))
            tmpp = ctx.enter_context(tc.tile_pool(name="tmp", bufs=2))
            outp = ctx.enter_context(tc.tile_pool(name="outp", bufs=3))
            cps = ctx.enter_context(tc.tile_pool(name="cps", bufs=2, space="PSUM"))
            k8ps = ctx.enter_context(tc.tile_pool(name="k8ps", bufs=1, space="PSUM"))
            tps = ctx.enter_context(tc.tile_pool(name="tps", bufs=2, space="PSUM"))
            aps = ctx.enter_context(tc.tile_pool(name="aps", bufs=3, space="PSUM"))

            wb_sb = consts.tile([128, 160], F16, tag="wb")
            biasc = consts.tile([48, 2], F32, tag="biasc")
            nc.scalar.dma_start(out=biasc[:, :], in_=biasc_d[:, :])
            nc.scalar.dma_start(out=wb_sb[:, :], in_=wblk_d[:, :])
            tm_sb = consts.tile([108, 2 * CPC * 3 * P2], F32R, tag="tm")
            nc.scalar.dma_start(out=tm_sb[:, :], in_=tm_d[:, :])
            tm_v = tm_sb.rearrange("p (t c k j) -> p t c k j", t=2, c=CPC, k=3)

            xc3 = []
            px_sb = []
            for t in range(2):
                xc3.append(consts.tile([108, ND, HWF], F32R, tag=f"xc{t}",
                                       name=f"xc{t}"))
                px_sb.append([consts.tile([P2, L], F16, tag=f"px{t}{c}",
                                          name=f"px{t}{c}")
                              for c in range(CPC)])

            # transform helper: z = sum_p TM[t,c,p].T @ xc3[:, pd-pair, p::9]
            def transform(t, c, ch):
                zp = tps.tile([P2, 512], F32, tag="tps")
                for p in range(3):
                    rhs = xc3[t][0:108, 2 * ch: 2 * ch + 2, p:HWF:P]
                    nc.tensor.matmul(
                        zp[:, :], tm_v[:, t, c, p, :], rhs,
                        start=(p == 0), stop=(p == 2))
                zm = tmpp.tile([P2, 512], F32, tag="zm")
                nc.scalar.mul(zm[:, :], zp[:, :], 0.2)
                nc.vector.tensor_tensor(
                    out=px_sb[t][c][:, ch * 512: ch * 512 + 512],
                    in0=zp[:, :], in1=zm[:, :],
                    op=mybir.AluOpType.max)

            def att_c(c):
                # att[c] = pxT[c].T @ pyT[c]
                for m in range(8):  # l1 chunks of 128
                    ob = outp.tile([128, L], F16, tag="ob")
                    for nch in range(2):  # l2 chunks of 512
                        ap_ = aps.tile([128, 512], F32, tag="aps")
                        nc.tensor.matmul(
                            ap_[:, :],
                            px_sb[0][c][:, m * 128: m * 128 + 128],
                            px_sb[1][c][:, nch * 512: nch * 512 + 512],
                            start=True, stop=True)
                        dst = ob[:, nch * 512: nch * 512 + 512]
                        if nch % 2 == 0:
                            nc.vector.tensor_scalar_mul(dst, ap_[:, :],
                                                        1.0 / P2)
                        else:
                            nc.scalar.mul(dst, ap_[:, :], 1.0 / P2)
                    nc.sync.dma_start(
                        out=att_d[c, m * 128: m * 128 + 128, :], in_=ob[:, :])

            ncopy = 0
            for t in range(2):
                src = x_d if t == 0 else y_d
                xb8 = xb8p.tile([128, HWF], F16, tag="xb8")
                xk8 = xb8p.tile([16, HWF], F32R, tag="xk8")
                xbs = []
                for pd in range(ND):
                    xb = xbp.tile([128, 2, HWF], F16, tag="xb")
                    for i in range(2):
                        d0 = 9 * pd + 4 * i
                        nc.sync.dma_start(out=xb[:, i, :],
                                          in_=src[:, d0: d0 + 4, :])
                    xbs.append(xb)
                    if pd == 0:
                        # kd=8 rows for all pd: partitions (c, pd)
                        nc.sync.dma_start(out=xb8[:, :], in_=src[:, 8::9, :])
                for pd in range(ND):
                    xb = xbs[pd]
                    for h0, hn in _HW_CHUNKS:
                        ps = cps.tile([32, 512], F32, tag="cps")
                        for i in range(2):
                            lhs = wb_sb[0:128,
                                        (t * 2 + i) * 32:(t * 2 + i + 1) * 32]
                            nc.tensor.matmul(ps[:, :hn], lhs, xb[:, i, h0: h0 + hn],
                                             start=(i == 0), stop=(i == 1))
                        dst = xc3[t][0:32, pd, h0: h0 + hn]
                        if ncopy % 2 == 0:
                            nc.vector.tensor_scalar_add(dst, ps[:, :hn],
                                                        biasc[0:32, t: t + 1])
                        else:
                            nc.scalar.activation(dst, ps[:, :hn], IDENT,
                                                 bias=biasc[0:32, t: t + 1])
                        ncopy += 1
                    if pd == 0:
                        # kd=8 for all pd in one K=128 pass: out rows (pd, o)
                        for h0, hn in _HW_CHUNKS:
                            ps8 = k8ps.tile([16, 512], F32, tag="k8")
                            nc.tensor.matmul(
                                ps8[:, :hn],
                                wb_sb[0:128, 128 + t * 16: 144 + t * 16],
                                xb8[:, h0: h0 + hn], start=True, stop=True)
                            dst = xk8[:, h0: h0 + hn]
                            if ncopy % 2 == 0:
                                nc.vector.tensor_scalar_add(
                                    dst, ps8[:, :hn], biasc[32:48, t: t + 1])
                            else:
                                nc.scalar.activation(dst, ps8[:, :hn], IDENT,
                                                     bias=biasc[32:48, t: t + 1])
                            ncopy += 1
                        for pdd in range(ND):
                            nc.gpsimd.dma_start(
                                out=xc3[t][32:36, pdd, :],
                                in_=xk8[4 * pdd: 4 * pdd + 4, :])
                    # replicate into kw-shifted row groups (rows 36-71, 72-107)
                    # on the SWDGE queue so it can't head-of-line-block loads
                    for g in (1, 2):
                        s = 3 * g
                        nc.gpsimd.dma_start(
                            out=xc3[t][36 * g: 36 * g + 36, pd, 0: HWF - s],
                            in_=xc3[t][0:36, pd, s:HWF])
                    if pd % 2 == 1:
                        # l-chunk (pd-1, pd) complete: run its transforms now
                        # so the PE has work while the next loads stream in
                        for c in range(CPC):
                            transform(t, c, pd // 2)
                            if t == 1 and pd == 3:
                                att_c(c)

    nc.compile()
    _CACHE["nc"] = nc
    return nc


def _host_prep(x, y, W_img, b_img, W_fea, b_fea, W1, W2):
    """Build per-core wblk / tm / bias arrays. Returns in_maps list."""
    x = np.ascontiguousarray(
        np.asarray(x, np.float32).reshape(C, D, HWF).astype(np.float16))
    y = np.ascontiguousarray(
        np.asarray(y, np.float32).reshape(C, D, HWF).astype(np.float16))
    W_img = np.asarray(W_img, np.float32)
    b_img = np.asarray(b_img, np.float32)
    W_fea = np.asarray(W_fea, np.float32)
    b_fea = np.asarray(b_fea, np.float32)
    A = np.asarray(W2, np.float32) @ np.asarray(W1, np.float32)  # (81, 81)

    in_maps = []
    for r in range(N_CORES):
        Wl = [W_img[r * CPC:(r + 1) * CPC, :], W_fea[r * CPC:(r + 1) * CPC, :]]
        bl = [b_img[r * CPC:(r + 1) * CPC], b_fea[r * CPC:(r + 1) * CPC]]

        # conv lhsT: passes i<2 rows p=c*4+kd_l (kd=4i+kd_l), kd8 block
        wblk = np.zeros((128, 160), np.float32)
        for t in range(2):
            for i in range(2):
                for kd_l in range(4):
                    kd = 4 * i + kd_l
                    for o in range(CPC):
                        col = (t * 2 + i) * 32 + kd * 4 + o
                        wblk[kd_l::4, col] = Wl[t][o, :]  # rows c*4+kd_l
            for pd in range(ND):
                for o in range(CPC):
                    wblk[pd::4, 128 + t * 16 + pd * 4 + o] = Wl[t][o, :]

        biasc = np.zeros((48, 2), np.float32)
        for t in range(2):
            biasc[0:32, t] = np.tile(bl[t], 8)
            biasc[32:48, t] = np.tile(bl[t], 4)

        # tm3[p, t, c, pass, j]; p = g*36 + kd*4 + o, kw = pass + 3g
        tm = np.zeros((108, 2, CPC, 3, P2), np.float32)
        for g in range(3):
            for kd in range(P):
                for o in range(CPC):
                    row = g * 36 + kd * 4 + o
                    for pp in range(3):
                        kw = pp + 3 * g
                        tm[row, :, o, pp, :] = A[:, kd * P + kw]
        tm = tm.reshape(108, 2 * CPC * 3 * P2)

        in_maps.append({
            "x": x, "y": y,
            "wblk": wblk.astype(np.float16),
            "tm": np.ascontiguousarray(tm),
            "biasc": biasc,
        })
    return in_maps


def kernel(**inputs):
    global last_results
    nc = _build()
    in_maps = _host_prep(**inputs)
    trace = bool(os.environ.get("KERNEL_TRACE"))
    res = run_bass_kernel_spmd(nc, in_maps, core_ids=list(range(N_CORES)),
                               trace=trace)
    last_results = res
    att = np.stack([res.results[r]["att"] for r in range(N_CORES)])
    return att.reshape(1, C, L, L).astype(np.float32)
